# revision 10
# baseline (speedup 1.0000x reference)
"""CapsuleLayer (dynamic routing) Trainium2 kernel, v3.

Problem: B=128, I=1152 input capsules (A=8), O=10 output capsules (OA=16),
3 routing iterations.  Data-parallel over batch: 8 cores x 16 examples.

Per-core layout: SBUF partition p = is*16 + b (is = i mod 8, b = local batch),
chunk c = i // 8 in the free dim, vote coordinate n = oa*10 + o.

v3 structure (vs v2):
  - Unpaired votes matmuls: contraction over q=(is,a) (64 partitions) per
    single half-chunk c, f=160.  Kills the 2x h-doubling of the weights
    (DMA 8.3MB -> 5.5MB) and gives s1 a single accumulator chain.
  - s1 matmuls run ahead of the votes matmuls (LAG subwaves) so squash1 /
    vrep / the t=1 tmp multiplies overlap the votes-matmul tail.
  - PSUM->SBUF votes copies split across Scalar/Pool/Vector with 4 PSUM
    banks so the PE never stalls on copies.
  - Big elementwise multiplies (wv = route*votes, tmp = votes*vrep) split
    between DVE and GpSimd (Pool).
  - sqrt(x) = exp(0.5*ln(x)): every scalar activation (Exp/Ln/Copy) lives
    in act-table set 6, so zero mid-kernel ACT_TABLE_LOADs; set is warmed
    by dummy Ln+Exp at t=0 under the input DMAs.
  - delta (logits update) via accumulating identity matmuls per half
    c-block (f=240) pipelined behind the tmp waves.
"""

import numpy as np
import ml_dtypes

B, I, A, O, OA = 128, 1152, 8, 10, 16
NCORES = 8
BL = B // NCORES        # 16 examples per core
IS8 = 8                 # i-positions per chunk
C = I // IS8            # 144 chunks
Q = IS8 * A             # 64 contraction rows (is, a)
N = O * OA              # 160, n = oa*O + o
P = 128                 # p = is*BL + b
NUM_ROUTING = 3
CBN = 3                 # logits c-blocks (PSUM banks)
CBS = C // CBN          # 48 chunks per block
HW2 = CBS // 2          # 24 chunks per half block
SW = 3                  # chunks per s-matmul
GRP3 = 3                # chunks per psum copy group
NSUB = C // GRP3        # 48 subwaves / copy groups
LAG = 12                # votes-mm lag behind s1-mm, in subwaves
NDMA = 12               # input DMA waves
DSTEP = C // NDMA

# per-group copy engine pattern (S=scalar, V=vector; GPSIMD can't read PSUM)
COPY_PAT = ["S", "V", "S"]
# DVE chunk share per 24-chunk wave for the big multiplies (rest -> Pool)
T1_SPLIT = [19, 19, 19, 19, 19, 19]   # t=1 tmp waves
WV_SPLIT = [19, 19, 19, 19, 19, 19]   # t>=2 wv waves
TMP_SPLIT = [19, 19, 19, 19, 19, 19]  # t>=2 tmp waves

_NC_CACHE = {}


def _build_nc():
    from contextlib import ExitStack

    import concourse.tile as tile
    import concourse.mybir as mybir
    from concourse import bacc

    F32 = mybir.dt.float32
    BF16 = mybir.dt.bfloat16
    AF = mybir.ActivationFunctionType
    ALU = mybir.AluOpType
    AX = mybir.AxisListType

    nc = bacc.Bacc()
    xbd_d = nc.dram_tensor("xbd", [Q, C, P], BF16, kind="ExternalInput")
    xfl_d = nc.dram_tensor("xfl", [Q, C, BL], BF16, kind="ExternalInput")
    w1c_d = nc.dram_tensor("w1c", [Q, C, N], BF16, kind="ExternalInput")
    bsel_d = nc.dram_tensor("bsel", [P, BL], BF16, kind="ExternalInput")
    brep_d = nc.dram_tensor("brep", [BL, P], BF16, kind="ExternalInput")
    bias_d = nc.dram_tensor("biasr", [BL, N], F32, kind="ExternalInput")
    id_d = nc.dram_tensor("id128", [P, P], BF16, kind="ExternalInput")
    vout_d = nc.dram_tensor("vout", [BL, N], F32, kind="ExternalOutput")

    with ExitStack() as ctx:
        tc = ctx.enter_context(tile.TileContext(nc))
        st = ctx.enter_context(tc.tile_pool(name="static", bufs=1))
        itp = ctx.enter_context(tc.tile_pool(name="itp", bufs=1))
        pps = ctx.enter_context(tc.tile_pool(name="pps", bufs=1, space="PSUM"))

        w1c = st.tile([Q, C, N], BF16)
        xbd = st.tile([Q, C, P], BF16)
        xfl = st.tile([Q, C, BL], BF16)
        votes = st.tile([P, C, N], BF16)
        big = st.tile([P, C, N], BF16)      # shared wv/tmp buffer
        expb = st.tile([P, C, O], BF16)
        route = st.tile([P, C, O], BF16)
        z = st.tile([P, C], F32)
        rz = st.tile([P, C], F32)
        bsel = st.tile([P, BL], BF16)
        brep = st.tile([BL, P], BF16)
        biasr = st.tile([BL, N], F32)
        id128 = st.tile([P, P], BF16)

        # PSUM (bank = 512 f32): 3 logits banks + 1 s bank (persistent)
        lg0 = pps.tile([P, 512], F32, tag="lg0")
        lg1 = pps.tile([P, 512], F32, tag="lg1")
        lg2 = pps.tile([P, 512], F32, tag="lg2")
        lg = [lg0, lg1, lg2]
        s_ps = pps.tile([BL, 512], F32, tag="sps")

        dz = st.tile([1, 1], F32)
        dl = st.tile([1, 1], F32)

        v4 = votes[:].rearrange("p c (oa o) -> p c oa o", o=O)
        b4 = big[:].rearrange("p c (oa o) -> p c oa o", o=O)

        def big_mul(lo, nv, hi, dst, src0, src1):
            """dst[:, lo:hi] = src0*src1, DVE on [lo, lo+nv), Pool on rest."""
            if nv > 0:
                nc.vector.tensor_mul(
                    dst[:, lo : lo + nv], src0[:, lo : lo + nv],
                    src1[:, lo : lo + nv],
                )
            if lo + nv < hi:
                nc.gpsimd.tensor_mul(
                    dst[:, lo + nv : hi], src0[:, lo + nv : hi],
                    src1[:, lo + nv : hi],
                )

        def squash(t):
            """squash s -> v tile; s comes from s_ps.  Returns v tile."""
            s_t = itp.tile([BL, N], F32, tag="stile")
            if t == 1:
                # s_ps[b, n] = sum_i votes; route is uniform 1/O at t=1
                nc.vector.scalar_tensor_tensor(
                    s_t[:], s_ps[:, 0:N], 1.0 / O, biasr[:],
                    op0=ALU.mult, op1=ALU.add,
                )
            else:
                sa = itp.tile([BL, N], F32, tag="sa")
                nc.vector.reduce_sum(
                    sa[:],
                    s_ps[:, 0 : SW * N].rearrange("b (c n) -> b n c", c=SW),
                    axis=AX.X,
                )
                nc.vector.tensor_add(s_t[:], sa[:], biasr[:])
            sq = itp.tile([BL, N], F32, tag="sq")
            nc.vector.tensor_mul(sq[:], s_t[:], s_t[:])
            nsq = itp.tile([BL, OA], F32, tag="nsq")
            nc.vector.reduce_sum(
                nsq[:], sq[:].rearrange("b (oa o) -> b oa o", o=O), axis=AX.X
            )
            nsq1 = itp.tile([BL, OA], F32, tag="nsq1")
            nc.vector.tensor_scalar_add(nsq1[:], nsq[:], 1.0)
            rn1 = itp.tile([BL, OA], F32, tag="rn1")
            nc.vector.reciprocal_approx_fast(rn1[:], nsq1[:])
            # sqrt(nsq) = exp(0.5*ln(nsq)): stays in act table set 6
            lnn = itp.tile([BL, OA], F32, tag="lnn")
            nc.scalar.activation(lnn[:], nsq[:], AF.Ln)
            sr = itp.tile([BL, OA], F32, tag="sr")
            nc.scalar.activation(sr[:], lnn[:], AF.Exp, scale=0.5)
            f = itp.tile([BL, OA], F32, tag="f")
            nc.vector.tensor_mul(f[:], sr[:], rn1[:])
            vt = itp.tile([BL, N], F32 if t == NUM_ROUTING else BF16, tag="vt")
            nc.vector.tensor_mul(
                vt[:].rearrange("b (oa o) -> b oa o", o=O),
                s_t[:].rearrange("b (oa o) -> b oa o", o=O),
                f[:].unsqueeze(2).broadcast_to([BL, OA, O]),
            )
            return vt

        def replicate(vbf, vrep):
            vr_ps = piv.tile([P, 512], F32, tag="vrps")
            nc.tensor.matmul(
                vr_ps[:, 0:N], lhsT=brep[:], rhs=vbf[:], start=True, stop=True
            )
            nc.scalar.copy(vrep[:], vr_ps[:, 0:N])

        def delta_half(t, cb, h):
            """logits[cb] += sum_oa tmp for half h, via 16 id matmuls."""
            sl = slice(cb * CBS + h * HW2, cb * CBS + (h + 1) * HW2)
            dst = lg[cb][:, h * HW2 * O : (h + 1) * HW2 * O]
            for oa in range(OA):
                # start only on the bank's very first matmul: start=True marks
                # the WHOLE 2KB zero region pending-zero, so a second start
                # (h=1) would discard h=0's accumulated half.  h=1's first
                # write still zeroes its range via the pending-zero bits.
                nc.tensor.matmul(
                    dst,
                    lhsT=id128[:],
                    rhs=b4[:, sl, oa, :],
                    start=(t == 1 and oa == 0 and h == 0),
                    stop=(t == NUM_ROUTING - 1 and oa == OA - 1 and h == 1),
                    skip_group_check=True,
                )

        # ---- phase V: s1 matmuls ahead, votes matmuls + copies behind ----
        with tc.tile_pool(name="psv", bufs=4, space="PSUM") as psv:
            # warm act table set 6 (ln+exp) under the DMAs
            nc.vector.memset(dz[:], 1.0)
            nc.scalar.activation(dl[:], dz[:], AF.Ln)
            nc.scalar.activation(dl[:], dz[:], AF.Exp)
            nc.sync.dma_start(out=bsel[:], in_=bsel_d[:])
            nc.sync.dma_start(out=brep[:], in_=brep_d[:])
            nc.sync.dma_start(out=biasr[:], in_=bias_d[:])
            nc.sync.dma_start(out=id128[:], in_=id_d[:])
            for q in range(NDMA):
                sl = slice(q * DSTEP, (q + 1) * DSTEP)
                nc.sync.dma_start(out=xfl[:, sl, :], in_=xfl_d[:, sl, :])
                nc.sync.dma_start(out=w1c[:, sl, :], in_=w1c_d[:, sl, :])
                nc.sync.dma_start(out=xbd[:, sl, :], in_=xbd_d[:, sl, :])

            cp_eng = {
                "S": nc.scalar.copy,
                "P": nc.gpsimd.tensor_copy,
                "V": nc.vector.tensor_copy,
            }

            def votes_sub(g):
                ps = psv.tile([P, 512], F32, tag="pv")
                for j in range(GRP3):
                    c = g * GRP3 + j
                    nc.tensor.matmul(
                        ps[:, j * N : (j + 1) * N],
                        lhsT=xbd[:, c, :],
                        rhs=w1c[:, c, :],
                        start=True,
                        stop=True,
                    )
                src = ps[:, 0 : GRP3 * N]
                dst = votes[:, g * GRP3 : (g + 1) * GRP3, :].rearrange(
                    "p c n -> p (c n)"
                )
                cp_eng[COPY_PAT[g % len(COPY_PAT)]](dst, src)

            for k in range(NSUB):
                for j in range(GRP3):
                    c = k * GRP3 + j
                    nc.tensor.matmul(
                        s_ps[:, 0:N],
                        lhsT=xfl[:, c, :],
                        rhs=w1c[:, c, :],
                        start=(c == 0),
                        stop=(c == C - 1),
                    )
                if k >= LAG:
                    votes_sub(k - LAG)
            # ---- t=1 squash overlaps the votes-matmul tail; the vrep
            # matmul goes AFTER the flush so trailing votes matmuls don't
            # stall behind the squash dependency in the in-order PE queue
            v1 = squash(1)
            for g in range(NSUB - LAG, NSUB):
                votes_sub(g)
            vrep = itp.tile([P, N], BF16, tag="vrep")
            vr_ps1 = psv.tile([P, 512], F32, tag="pv")
            nc.tensor.matmul(
                vr_ps1[:, 0:N], lhsT=brep[:], rhs=v1[:], start=True, stop=True
            )
            nc.scalar.copy(vrep[:], vr_ps1[:, 0:N])

        piv = ctx.enter_context(tc.tile_pool(name="piv", bufs=1, space="PSUM"))

        # ---- t=1: tmp = votes*vrep + delta into logits PSUM ----
        vr_b = vrep[:].unsqueeze(1).broadcast_to([P, C, N])
        for w in range(2 * CBN):
            lo = w * HW2
            big_mul(lo, T1_SPLIT[w], lo + HW2, big[:], votes[:], vr_b)
            delta_half(1, w // 2, w % 2)

        # ---- routing iterations t=2..3 ----
        for t in range(2, NUM_ROUTING + 1):
            # softmax over o from PSUM-resident logits, per c-block
            r4 = route[:].unsqueeze(2).broadcast_to([P, C, OA, O])
            for cb in range(CBN):
                sl = slice(cb * CBS, (cb + 1) * CBS)
                src = lg[cb][:, 0 : CBS * O].rearrange("p (c o) -> p c o", o=O)
                nc.scalar.activation(expb[:, sl], src, AF.Exp)
                nc.vector.reduce_sum(z[:, sl], expb[:, sl], axis=AX.X)
                nc.vector.reciprocal_approx_fast(rz[:, sl], z[:, sl])
                nc.vector.tensor_mul(
                    route[:, sl],
                    expb[:, sl],
                    rz[:, sl].unsqueeze(2).broadcast_to([P, CBS, O]),
                )
                # wv = route*votes, two 24-chunk waves per block, then the
                # s-chain matmuls for those chunks
                for h in range(2):
                    w = cb * 2 + h
                    lo = w * HW2
                    big_mul(lo, WV_SPLIT[w], lo + HW2, b4, v4, r4)
                    for j in range(lo // SW, (lo + HW2) // SW):
                        rhs = big[:, j * SW : (j + 1) * SW, :].rearrange(
                            "p c n -> p (c n)"
                        )
                        nc.tensor.matmul(
                            s_ps[:, 0 : SW * N],
                            lhsT=bsel[:],
                            rhs=rhs,
                            start=(j == 0),
                            stop=(j == C // SW - 1),
                        )

            vt = squash(t)
            if t == NUM_ROUTING:
                nc.sync.dma_start(out=vout_d[:], in_=vt[:])
                break

            vrep2 = itp.tile([P, N], BF16, tag="vrep2")
            replicate(vt, vrep2)

            # tmp = votes*vrep + delta into logits PSUM
            vr_b2 = vrep2[:].unsqueeze(1).broadcast_to([P, C, N])
            for w in range(2 * CBN):
                lo = w * HW2
                big_mul(lo, TMP_SPLIT[w], lo + HW2, big[:], votes[:], vr_b2)
                delta_half(t, w // 2, w % 2)

    nc.compile()
    return nc


def get_nc():
    if "nc" not in _NC_CACHE:
        _NC_CACHE["nc"] = _build_nc()
    return _NC_CACHE["nc"]


def make_in_maps(x, weights, biases):
    bf = ml_dtypes.bfloat16
    x = np.asarray(x, np.float32)
    weights = np.asarray(weights, np.float32)
    biases = np.asarray(biases, np.float32)

    # wn[i, a, oa*O + o] = weights[i, a, o*OA + oa]
    wn = (
        weights.reshape(I, A, O, OA).transpose(0, 1, 3, 2).reshape(I, A, N)
    )
    # w1c[(is, a), c, n] = wn[c*8+is, a, n]
    w1c = (
        wn.reshape(C, IS8, A, N).transpose(1, 2, 0, 3).reshape(Q, C, N)
    ).astype(bf)

    eye = np.eye(BL, dtype=np.float32)
    bsel = np.tile(eye, (IS8, 1)).astype(bf)  # bsel[p, b'] = delta(p % BL == b')
    brep = np.tile(eye, (1, IS8)).astype(bf)  # brep[b, p] = delta(b == p % BL)
    biasr = np.broadcast_to(biases.T.reshape(1, N), (BL, N)).astype(np.float32).copy()
    id128 = np.eye(P, dtype=np.float32).astype(bf)

    in_maps = []
    idx = np.arange(IS8)
    for k in range(NCORES):
        xc = x[k * BL : (k + 1) * BL]  # [BL, I, A]
        xx = xc.reshape(BL, C, IS8, A).transpose(2, 3, 1, 0)  # [is, a, c, b]
        xfl = xx.reshape(Q, C, BL).astype(bf)
        xbd = np.zeros((IS8, A, C, IS8, BL), np.float32)
        xbd[idx, :, :, idx, :] = xx[idx]
        xbd = xbd.reshape(Q, C, P).astype(bf)
        in_maps.append(
            {
                "xbd": np.ascontiguousarray(xbd),
                "xfl": np.ascontiguousarray(xfl),
                "w1c": np.ascontiguousarray(w1c),
                "bsel": bsel,
                "brep": brep,
                "biasr": biasr,
                "id128": id128,
            }
        )
    return in_maps


def assemble_out(results):
    out = np.zeros((B, 1, O, OA), np.float32)
    for k in range(NCORES):
        v = np.asarray(results[k]["vout"], np.float32)  # [BL, N], n = oa*O + o
        out[k * BL : (k + 1) * BL, 0] = v.reshape(BL, OA, O).transpose(0, 2, 1)
    return out


def kernel(x, weights, biases):
    from concourse.bass_utils import run_bass_kernel_spmd

    nc = get_nc()
    in_maps = make_in_maps(x, weights, biases)
    res = run_bass_kernel_spmd(nc, in_maps, list(range(NCORES)))
    return assemble_out(res.results)


# revision 13
# speedup vs baseline: 1.2344x; 1.2344x over previous
"""CapsuleLayer (dynamic routing) Trainium2 kernel, v4.

Problem: B=128, I=1152 input capsules (A=8), O=10 output capsules (OA=16),
3 routing iterations.  Data-parallel over batch: 8 cores x 16 examples.

Per-core layout: SBUF partition p = is*16 + b (is = i mod 8, b = local batch),
half-chunk c = i // 8 in the free dim, vote coordinate n = oa*10 + o.
Phase V contracts over q = (h, is, a) (128 rows) per PAIR of half-chunks
(f=320 per matmul so the ~120ns LDWEIGHTS hides under the stream).

v4 (vs v2 baseline):
  - Compact weight DMA: w2c's block-diagonal zero halves are memset by the
    idle GpSimd engine; DMA ships only the 2.95MB compact weights (two
    strided halves) instead of the 5.9MB doubled tensor.  xbd is built on
    DVE as xfl*mask instead of DMAing the 16x-inflated 2.36MB tensor.
    Total input DMA: 8.4MB -> 3.2MB.
  - s1 matmuls run LAG pair-chunks ahead of the votes matmuls so squash1 /
    vrep / t=1 tmp start as early as possible.
  - PSUM->SBUF votes copies on Scalar(2/3) + Vector(1/3) with 2x2-bank
    PSUM buffers; no PE stalls.
  - Squash needs no Scalar tables: sqrt via DVE fast-inverse-sqrt
    (bit-trick seed + 1 Newton step).  Only Exp (softmax) and Copy (vrep)
    remain on Scalar -> a single ACT_TABLE_LOAD at t=0, warmed by a dummy.
  - No GpSimd elementwise offload: Pool TENSOR_TENSOR is ~3.9ns/el AND
    SBUF port contention halves DVE to 1x when they overlap (measured).
"""

import numpy as np
import ml_dtypes

B, I, A, O, OA = 128, 1152, 8, 10, 16
NCORES = 8
BL = B // NCORES        # 16 examples per core
IS8 = 8                 # i-positions per half-chunk
C = I // IS8            # 144 half-chunks
CP = C // 2             # 72 paired chunks
N = O * OA              # 160, n = oa*O + o
N2 = 2 * N              # 320 per paired chunk
P = 128                 # p = is*BL + b
NUM_ROUTING = 3
CBN = 3                 # logits c-blocks (PSUM banks)
CBS = C // CBN          # 48 half-chunks per block
HW2 = CBS // 2          # 24 half-chunks per wave
SW = 3                  # half-chunks per s-matmul
LAG = 12                # votes-mm lag behind s1-mm, in paired chunks
NDMA = 12               # input DMA waves
DSTEP = CP // NDMA      # 6 paired chunks per wave

MAGIC = 0x5F3759DF      # fast inverse sqrt seed

_NC_CACHE = {}


def _build_nc():
    from contextlib import ExitStack

    import concourse.tile as tile
    import concourse.mybir as mybir
    from concourse import bacc

    F32 = mybir.dt.float32
    I32 = mybir.dt.int32
    BF16 = mybir.dt.bfloat16
    AF = mybir.ActivationFunctionType
    ALU = mybir.AluOpType
    AX = mybir.AxisListType

    nc = bacc.Bacc()
    wc_d = nc.dram_tensor("wc", [P, CP, N], BF16, kind="ExternalInput")
    xfl_d = nc.dram_tensor("xfl", [P, CP, BL], BF16, kind="ExternalInput")
    mask_d = nc.dram_tensor("mask", [P, P], BF16, kind="ExternalInput")
    bsel_d = nc.dram_tensor("bsel", [P, BL], BF16, kind="ExternalInput")
    brep_d = nc.dram_tensor("brep", [BL, P], BF16, kind="ExternalInput")
    bias_d = nc.dram_tensor("biasr", [BL, N], F32, kind="ExternalInput")
    id_d = nc.dram_tensor("id128", [P, P], BF16, kind="ExternalInput")
    vout_d = nc.dram_tensor("vout", [BL, N], F32, kind="ExternalOutput")

    with ExitStack() as ctx:
        tc = ctx.enter_context(tile.TileContext(nc))
        st = ctx.enter_context(tc.tile_pool(name="static", bufs=1))
        itp = ctx.enter_context(tc.tile_pool(name="itp", bufs=1))
        pps = ctx.enter_context(tc.tile_pool(name="pps", bufs=1, space="PSUM"))

        w2c = st.tile([P, CP, N2], BF16)
        xbd = st.tile([P, CP, P], BF16)
        xfl = st.tile([P, CP, BL], BF16)
        mask = st.tile([P, P], BF16)
        votes = st.tile([P, C, N], BF16)
        big = st.tile([P, C, N], BF16)      # shared wv/tmp buffer
        expb = st.tile([P, C, O], BF16)
        route = st.tile([P, C, O], BF16)
        z = st.tile([P, C], F32)
        rz = st.tile([P, C], F32)
        bsel = st.tile([P, BL], BF16)
        brep = st.tile([BL, P], BF16)
        biasr = st.tile([BL, N], F32)
        id128 = st.tile([P, P], BF16)
        sh1 = st.tile([BL, 1], I32)
        negm = st.tile([BL, 1], I32)
        magic = st.tile([BL, 1], I32)
        dz = st.tile([1, 1], F32)
        dl = st.tile([1, 1], F32)

        # PSUM (bank = 512 f32): 3 logits banks + 1 s bank (persistent)
        lg0 = pps.tile([P, 512], F32, tag="lg0")
        lg1 = pps.tile([P, 512], F32, tag="lg1")
        lg2 = pps.tile([P, 512], F32, tag="lg2")
        lg = [lg0, lg1, lg2]
        s_ps = pps.tile([BL, 512], F32, tag="sps")

        v4 = votes[:].rearrange("p c (oa o) -> p c oa o", o=O)
        b4 = big[:].rearrange("p c (oa o) -> p c oa o", o=O)

        def squash(t):
            """squash s -> v tile (DVE only: fast-inverse-sqrt, no tables)."""
            s_t = itp.tile([BL, N], F32, tag="stile")
            if t == 1:
                # s1 psum holds the two i-parity halves side by side
                # (one PSUM operand max per DVE op: copy one half out first)
                sa = itp.tile([BL, N], F32, tag="sa")
                nc.vector.tensor_copy(sa[:], s_ps[:, 0:N])
                nc.vector.tensor_add(sa[:], sa[:], s_ps[:, N:N2])
                nc.vector.scalar_tensor_tensor(
                    s_t[:], sa[:], 1.0 / O, biasr[:], op0=ALU.mult, op1=ALU.add
                )
            else:
                sa = itp.tile([BL, N], F32, tag="sa")
                nc.vector.reduce_sum(
                    sa[:],
                    s_ps[:, 0 : SW * N].rearrange("b (c n) -> b n c", c=SW),
                    axis=AX.X,
                )
                nc.vector.tensor_add(s_t[:], sa[:], biasr[:])
            sq = itp.tile([BL, N], F32, tag="sq")
            nc.vector.tensor_mul(sq[:], s_t[:], s_t[:])
            nsq = itp.tile([BL, OA], F32, tag="nsq")
            nc.vector.reduce_sum(
                nsq[:], sq[:].rearrange("b (oa o) -> b oa o", o=O), axis=AX.X
            )
            nsq1 = itp.tile([BL, OA], F32, tag="nsq1")
            nc.vector.tensor_scalar_add(nsq1[:], nsq[:], 1.0)
            rn1 = itp.tile([BL, OA], F32, tag="rn1")
            nc.vector.reciprocal_approx_fast(rn1[:], nsq1[:])
            # sqrt(nsq) = nsq * rsqrt(nsq), rsqrt via bit-trick + 1 NR step
            # (int scalars aren't allowed on tensor_scalar, so the shift and
            # subtract use broadcast [BL,1] int tiles through tensor_tensor)
            hb = itp.tile([BL, OA], I32, tag="hb")
            nc.vector.tensor_tensor(
                hb[:], nsq[:].bitcast(I32),
                sh1[:].broadcast_to([BL, OA]),
                op=ALU.arith_shift_right,
            )
            y0 = itp.tile([BL, OA], I32, tag="y0")
            nc.vector.tensor_tensor(
                y0[:], magic[:].broadcast_to([BL, OA]), hb[:], op=ALU.subtract
            )
            y0f = y0[:].bitcast(F32)
            tt = itp.tile([BL, OA], F32, tag="tt")
            nc.vector.tensor_mul(tt[:], nsq[:], y0f)
            nc.vector.tensor_mul(tt[:], tt[:], y0f)
            uu = itp.tile([BL, OA], F32, tag="uu")
            nc.vector.tensor_scalar(
                uu[:], tt[:], -0.5, 1.5, op0=ALU.mult, op1=ALU.add
            )
            y1 = itp.tile([BL, OA], F32, tag="y1")
            nc.vector.tensor_mul(y1[:], y0f, uu[:])
            sr = itp.tile([BL, OA], F32, tag="sr")
            nc.vector.tensor_mul(sr[:], nsq[:], y1[:])
            f = itp.tile([BL, OA], F32, tag="f")
            nc.vector.tensor_mul(f[:], sr[:], rn1[:])
            vt = itp.tile([BL, N], F32 if t == NUM_ROUTING else BF16, tag="vt")
            nc.vector.tensor_mul(
                vt[:].rearrange("b (oa o) -> b oa o", o=O),
                s_t[:].rearrange("b (oa o) -> b oa o", o=O),
                f[:].unsqueeze(2).broadcast_to([BL, OA, O]),
            )
            return vt

        def delta(t, cb):
            """logits[cb] += sum_oa tmp via 16 accumulating id matmuls."""
            sl = slice(cb * CBS, (cb + 1) * CBS)
            dst = lg[cb][:, 0 : CBS * O]
            for oa in range(OA):
                nc.tensor.matmul(
                    dst,
                    lhsT=id128[:],
                    rhs=b4[:, sl, oa, :],
                    start=(t == 1 and oa == 0),
                    stop=(t == NUM_ROUTING - 1 and oa == OA - 1),
                    skip_group_check=True,
                )

        # ---- phase V ----
        with tc.tile_pool(name="psv", bufs=2, space="PSUM") as psv:
            # constants + act-table warm (Exp -> set 0, which also has Copy)
            nc.vector.memset(dz[:], 1.0)
            nc.scalar.activation(dl[:], dz[:], AF.Exp)
            nc.vector.memset(sh1[:], 1)
            nc.vector.memset(negm[:], -1)
            nc.vector.memset(magic[:], MAGIC)
            nc.sync.dma_start(out=mask[:], in_=mask_d[:])
            nc.sync.dma_start(out=bsel[:], in_=bsel_d[:])
            nc.sync.dma_start(out=brep[:], in_=brep_d[:])
            nc.sync.dma_start(out=biasr[:], in_=bias_d[:])
            nc.sync.dma_start(out=id128[:], in_=id_d[:])
            for q in range(NDMA):
                sl = slice(q * DSTEP, (q + 1) * DSTEP)
                nc.sync.dma_start(out=xfl[:, sl, :], in_=xfl_d[:, sl, :])
                # compact weights into the diagonal blocks; GpSimd zeroes
                # the off-diagonal halves (disjoint ranges, fully parallel)
                nc.sync.dma_start(
                    out=w2c[0:64, sl, 0:N], in_=wc_d[0:64, sl, :]
                )
                nc.sync.dma_start(
                    out=w2c[64:P, sl, N:N2], in_=wc_d[64:P, sl, :]
                )
                nc.gpsimd.memset(w2c[0:64, sl, N:N2], 0.0)
                nc.gpsimd.memset(w2c[64:P, sl, 0:N], 0.0)
                # xbd wave: xfl broadcast over is' times the static mask
                nc.vector.tensor_mul(
                    xbd[:, sl, :].rearrange("p c (i b) -> p c i b", i=IS8),
                    xfl[:, sl, :].unsqueeze(2).broadcast_to(
                        [P, DSTEP, IS8, BL]
                    ),
                    mask[:].rearrange("p (i b) -> p i b", i=IS8)
                    .unsqueeze(1)
                    .broadcast_to([P, DSTEP, IS8, BL]),
                )

            def votes_pair(cp):
                # two votes matmuls per 2-bank psum tile, then one copy
                if cp % 2 == 0:
                    votes_pair.ps = psv.tile([P, 1024], F32, tag="pv")
                ps = votes_pair.ps
                j = cp % 2
                nc.tensor.matmul(
                    ps[:, j * 512 : j * 512 + N2],
                    lhsT=xbd[:, cp, :],
                    rhs=w2c[:, cp, :],
                    start=True,
                    stop=True,
                )
                if cp % 2 == 1:
                    g = cp // 2
                    src = ps[:].rearrange("p (j s) -> p j s", j=2)[:, :, 0:N2]
                    dst = votes[:, (cp - 1) * 2 : (cp + 1) * 2, :].rearrange(
                        "p (j c2) n -> p j (c2 n)", j=2
                    )
                    if g % 3 == 2:
                        nc.vector.tensor_copy(dst, src)
                    else:
                        nc.scalar.copy(dst, src)

            for cp in range(CP):
                nc.tensor.matmul(
                    s_ps[:, 0:N2],
                    lhsT=xfl[:, cp, :],
                    rhs=w2c[:, cp, :],
                    start=(cp == 0),
                    stop=(cp == CP - 1),
                )
                if cp >= LAG:
                    votes_pair(cp - LAG)
            v1 = squash(1)
            for cp in range(CP - LAG, CP):
                votes_pair(cp)
            vrep = itp.tile([P, N], BF16, tag="vrep")
            vr_ps1 = psv.tile([P, 1024], F32, tag="pv")
            nc.tensor.matmul(
                vr_ps1[:, 0:N], lhsT=brep[:], rhs=v1[:], start=True, stop=True
            )
            nc.scalar.copy(vrep[:], vr_ps1[:, 0:N])

        piv = ctx.enter_context(tc.tile_pool(name="piv", bufs=1, space="PSUM"))

        # ---- t=1: tmp = votes*vrep + delta into logits PSUM ----
        vr_b = vrep[:].unsqueeze(1).broadcast_to([P, C, N])
        for cb in range(CBN):
            for h in range(2):
                lo = cb * CBS + h * HW2
                nc.vector.tensor_mul(
                    big[:, lo : lo + HW2, :],
                    votes[:, lo : lo + HW2, :],
                    vr_b[:, lo : lo + HW2, :],
                )
            delta(1, cb)

        # ---- routing iterations t=2..3 ----
        for t in range(2, NUM_ROUTING + 1):
            r4 = route[:].unsqueeze(2).broadcast_to([P, C, OA, O])
            for cb in range(CBN):
                sl = slice(cb * CBS, (cb + 1) * CBS)
                src = lg[cb][:, 0 : CBS * O].rearrange("p (c o) -> p c o", o=O)
                nc.scalar.activation(expb[:, sl], src, AF.Exp)
                nc.vector.reduce_sum(z[:, sl], expb[:, sl], axis=AX.X)
                nc.vector.reciprocal_approx_fast(rz[:, sl], z[:, sl])
                nc.vector.tensor_mul(
                    route[:, sl],
                    expb[:, sl],
                    rz[:, sl].unsqueeze(2).broadcast_to([P, CBS, O]),
                )
                for h in range(2):
                    lo = cb * CBS + h * HW2
                    nc.vector.tensor_mul(
                        b4[:, lo : lo + HW2], v4[:, lo : lo + HW2],
                        r4[:, lo : lo + HW2],
                    )
                    for j in range(lo // SW, (lo + HW2) // SW):
                        rhs = big[:, j * SW : (j + 1) * SW, :].rearrange(
                            "p c n -> p (c n)"
                        )
                        nc.tensor.matmul(
                            s_ps[:, 0 : SW * N],
                            lhsT=bsel[:],
                            rhs=rhs,
                            start=(j == 0),
                            stop=(j == C // SW - 1),
                        )

            vt = squash(t)
            if t == NUM_ROUTING:
                nc.sync.dma_start(out=vout_d[:], in_=vt[:])
                break

            vrep2 = itp.tile([P, N], BF16, tag="vrep2")
            vr_ps = piv.tile([P, 512], F32, tag="vrps")
            nc.tensor.matmul(
                vr_ps[:, 0:N], lhsT=brep[:], rhs=vt[:], start=True, stop=True
            )
            nc.scalar.copy(vrep2[:], vr_ps[:, 0:N])

            vr_b2 = vrep2[:].unsqueeze(1).broadcast_to([P, C, N])
            for cb in range(CBN):
                for h in range(2):
                    lo = cb * CBS + h * HW2
                    nc.vector.tensor_mul(
                        big[:, lo : lo + HW2, :],
                        votes[:, lo : lo + HW2, :],
                        vr_b2[:, lo : lo + HW2, :],
                    )
                delta(t, cb)

    nc.compile()
    return nc


def get_nc():
    if "nc" not in _NC_CACHE:
        _NC_CACHE["nc"] = _build_nc()
    return _NC_CACHE["nc"]


def make_in_maps(x, weights, biases):
    bf = ml_dtypes.bfloat16
    x = np.asarray(x, np.float32)
    weights = np.asarray(weights, np.float32)
    biases = np.asarray(biases, np.float32)

    # wc[(h, is, a), cp, n=oa*O+o] = weights[(2cp+h)*8+is, a, o*OA+oa]
    w5 = (
        weights.reshape(CP, 2, IS8, A, O, OA)
        .transpose(0, 1, 2, 3, 5, 4)
        .reshape(CP, 2, IS8, A, N)
    )
    wc = np.ascontiguousarray(
        w5.transpose(1, 2, 3, 0, 4).reshape(P, CP, N)
    ).astype(bf)

    pidx = np.arange(P)
    is_p = (pidx // A) % IS8          # is of partition (h, is, a)
    is_c = np.arange(P) // BL         # is' of column (is', b)
    mask = (is_p[:, None] == is_c[None, :]).astype(np.float32).astype(bf)

    eye = np.eye(BL, dtype=np.float32)
    bsel = np.tile(eye, (IS8, 1)).astype(bf)  # bsel[p, b'] = delta(p % BL == b')
    brep = np.tile(eye, (1, IS8)).astype(bf)  # brep[b, p] = delta(b == p % BL)
    biasr = np.broadcast_to(biases.T.reshape(1, N), (BL, N)).astype(np.float32).copy()
    id128 = np.eye(P, dtype=np.float32).astype(bf)

    in_maps = []
    for k in range(NCORES):
        xc = x[k * BL : (k + 1) * BL]  # [BL, I, A]
        # xfl[(h, is, a), cp, b] = x[b, (2cp+h)*8+is, a]
        xfl = (
            xc.reshape(BL, CP, 2, IS8, A)
            .transpose(2, 3, 4, 1, 0)
            .reshape(P, CP, BL)
            .astype(bf)
        )
        in_maps.append(
            {
                "wc": wc,
                "xfl": np.ascontiguousarray(xfl),
                "mask": mask,
                "bsel": bsel,
                "brep": brep,
                "biasr": biasr,
                "id128": id128,
            }
        )
    return in_maps


def assemble_out(results):
    out = np.zeros((B, 1, O, OA), np.float32)
    for k in range(NCORES):
        v = np.asarray(results[k]["vout"], np.float32)  # [BL, N], n = oa*O + o
        out[k * BL : (k + 1) * BL, 0] = v.reshape(BL, OA, O).transpose(0, 2, 1)
    return out


def kernel(x, weights, biases):
    from concourse.bass_utils import run_bass_kernel_spmd

    nc = get_nc()
    in_maps = make_in_maps(x, weights, biases)
    res = run_bass_kernel_spmd(nc, in_maps, list(range(NCORES)))
    return assemble_out(res.results)


# revision 17
# speedup vs baseline: 1.2360x; 1.0013x over previous
"""CapsuleLayer (dynamic routing) Trainium2 kernel, v4.

Problem: B=128, I=1152 input capsules (A=8), O=10 output capsules (OA=16),
3 routing iterations.  Data-parallel over batch: 8 cores x 16 examples.

Per-core layout: SBUF partition p = is*16 + b (is = i mod 8, b = local batch),
half-chunk c = i // 8 in the free dim, vote coordinate n = oa*10 + o.
Phase V contracts over q = (h, is, a) (128 rows) per PAIR of half-chunks
(f=320 per matmul so the ~120ns LDWEIGHTS hides under the stream).

v4 (vs v2 baseline):
  - Compact weight DMA: w2c's block-diagonal zero halves are memset by the
    idle GpSimd engine; DMA ships only the 2.95MB compact weights (two
    strided halves) instead of the 5.9MB doubled tensor.  xbd is built on
    DVE as xfl*mask instead of DMAing the 16x-inflated 2.36MB tensor.
    Total input DMA: 8.4MB -> 3.2MB.
  - s1 matmuls run LAG pair-chunks ahead of the votes matmuls so squash1 /
    vrep / t=1 tmp start as early as possible.
  - PSUM->SBUF votes copies on Scalar(2/3) + Vector(1/3) with 2x2-bank
    PSUM buffers; no PE stalls.
  - Squash needs no Scalar tables: sqrt via DVE fast-inverse-sqrt
    (bit-trick seed + 1 Newton step).  Only Exp (softmax) and Copy (vrep)
    remain on Scalar -> a single ACT_TABLE_LOAD at t=0, warmed by a dummy.
  - No GpSimd elementwise offload: Pool TENSOR_TENSOR is ~3.9ns/el AND
    SBUF port contention halves DVE to 1x when they overlap (measured).
"""

import numpy as np
import ml_dtypes

B, I, A, O, OA = 128, 1152, 8, 10, 16
NCORES = 8
BL = B // NCORES        # 16 examples per core
IS8 = 8                 # i-positions per half-chunk
C = I // IS8            # 144 half-chunks
CP = C // 2             # 72 paired chunks
N = O * OA              # 160, n = oa*O + o
N2 = 2 * N              # 320 per paired chunk
P = 128                 # p = is*BL + b
NUM_ROUTING = 3
CBN = 3                 # logits c-blocks (PSUM banks)
CBS = C // CBN          # 48 half-chunks per block
HW2 = CBS // 2          # 24 half-chunks per wave
SW = 3                  # half-chunks per s-matmul
LAG = 12                # votes-mm lag behind s1-mm, in paired chunks
NDMA = 6                # input DMA waves
DSTEP = CP // NDMA      # 12 paired chunks per wave

MAGIC = 0x5F3759DF      # fast inverse sqrt seed

_NC_CACHE = {}


def _build_nc():
    from contextlib import ExitStack

    import concourse.tile as tile
    import concourse.mybir as mybir
    from concourse import bacc

    F32 = mybir.dt.float32
    I32 = mybir.dt.int32
    BF16 = mybir.dt.bfloat16
    AF = mybir.ActivationFunctionType
    ALU = mybir.AluOpType
    AX = mybir.AxisListType

    nc = bacc.Bacc()
    wc_d = nc.dram_tensor("wc", [P, CP, N], BF16, kind="ExternalInput")
    xfl_d = nc.dram_tensor("xfl", [P, CP, BL], BF16, kind="ExternalInput")
    mask_d = nc.dram_tensor("mask", [P, P], BF16, kind="ExternalInput")
    bsel_d = nc.dram_tensor("bsel", [P, BL], BF16, kind="ExternalInput")
    brep_d = nc.dram_tensor("brep", [BL, P], BF16, kind="ExternalInput")
    bias_d = nc.dram_tensor("biasr", [BL, N], F32, kind="ExternalInput")
    id_d = nc.dram_tensor("id128", [P, P], BF16, kind="ExternalInput")
    vout_d = nc.dram_tensor("vout", [BL, N], F32, kind="ExternalOutput")

    with ExitStack() as ctx:
        tc = ctx.enter_context(tile.TileContext(nc))
        st = ctx.enter_context(tc.tile_pool(name="static", bufs=1))
        itp = ctx.enter_context(tc.tile_pool(name="itp", bufs=1))
        pps = ctx.enter_context(tc.tile_pool(name="pps", bufs=1, space="PSUM"))

        w2c = st.tile([P, CP, N2], BF16)
        xbd = st.tile([P, CP, P], BF16)
        xfl = st.tile([P, CP, BL], BF16)
        mask = st.tile([P, P], BF16)
        votes = st.tile([P, C, N], BF16)
        big = st.tile([P, C, N], BF16)      # shared wv/tmp buffer
        expb = st.tile([P, C, O], BF16)
        route = st.tile([P, C, O], BF16)
        z = st.tile([P, C], F32)
        rz = st.tile([P, C], F32)
        bsel = st.tile([P, BL], BF16)
        brep = st.tile([BL, P], BF16)
        biasr = st.tile([BL, N], F32)
        id128 = st.tile([P, P], BF16)
        sh1 = st.tile([BL, 1], I32)
        negm = st.tile([BL, 1], I32)
        magic = st.tile([BL, 1], I32)
        dz = st.tile([1, 1], F32)
        dl = st.tile([1, 1], F32)

        # PSUM (bank = 512 f32): 3 logits banks + 1 s bank (persistent)
        lg0 = pps.tile([P, 512], F32, tag="lg0")
        lg1 = pps.tile([P, 512], F32, tag="lg1")
        lg2 = pps.tile([P, 512], F32, tag="lg2")
        lg = [lg0, lg1, lg2]
        s_ps = pps.tile([BL, 512], F32, tag="sps")

        v4 = votes[:].rearrange("p c (oa o) -> p c oa o", o=O)
        b4 = big[:].rearrange("p c (oa o) -> p c oa o", o=O)

        def squash(t):
            """squash s -> v tile (DVE only: fast-inverse-sqrt, no tables)."""
            s_t = itp.tile([BL, N], F32, tag="stile")
            if t == 1:
                # s1 psum holds the two i-parity halves side by side
                # (one PSUM operand max per DVE op: copy one half out first)
                sa = itp.tile([BL, N], F32, tag="sa")
                nc.vector.tensor_copy(sa[:], s_ps[:, 0:N])
                nc.vector.tensor_add(sa[:], sa[:], s_ps[:, N:N2])
                nc.vector.scalar_tensor_tensor(
                    s_t[:], sa[:], 1.0 / O, biasr[:], op0=ALU.mult, op1=ALU.add
                )
            else:
                sa = itp.tile([BL, N], F32, tag="sa")
                nc.vector.reduce_sum(
                    sa[:],
                    s_ps[:, 0 : SW * N].rearrange("b (c n) -> b n c", c=SW),
                    axis=AX.X,
                )
                nc.vector.tensor_add(s_t[:], sa[:], biasr[:])
            sq = itp.tile([BL, N], F32, tag="sq")
            nc.vector.tensor_mul(sq[:], s_t[:], s_t[:])
            nsq = itp.tile([BL, OA], F32, tag="nsq")
            nc.vector.reduce_sum(
                nsq[:], sq[:].rearrange("b (oa o) -> b oa o", o=O), axis=AX.X
            )
            nsq1 = itp.tile([BL, OA], F32, tag="nsq1")
            nc.vector.tensor_scalar_add(nsq1[:], nsq[:], 1.0)
            rn1 = itp.tile([BL, OA], F32, tag="rn1")
            nc.vector.reciprocal_approx_fast(rn1[:], nsq1[:])
            # sqrt(nsq) = nsq * rsqrt(nsq), rsqrt via bit-trick + 1 NR step
            # (int scalars aren't allowed on tensor_scalar, so the shift and
            # subtract use broadcast [BL,1] int tiles through tensor_tensor)
            hb = itp.tile([BL, OA], I32, tag="hb")
            nc.vector.tensor_tensor(
                hb[:], nsq[:].bitcast(I32),
                sh1[:].broadcast_to([BL, OA]),
                op=ALU.arith_shift_right,
            )
            y0 = itp.tile([BL, OA], I32, tag="y0")
            nc.vector.tensor_tensor(
                y0[:], magic[:].broadcast_to([BL, OA]), hb[:], op=ALU.subtract
            )
            y0f = y0[:].bitcast(F32)
            tt = itp.tile([BL, OA], F32, tag="tt")
            nc.vector.tensor_mul(tt[:], nsq[:], y0f)
            nc.vector.tensor_mul(tt[:], tt[:], y0f)
            uu = itp.tile([BL, OA], F32, tag="uu")
            nc.vector.tensor_scalar(
                uu[:], tt[:], -0.5, 1.5, op0=ALU.mult, op1=ALU.add
            )
            y1 = itp.tile([BL, OA], F32, tag="y1")
            nc.vector.tensor_mul(y1[:], y0f, uu[:])
            sr = itp.tile([BL, OA], F32, tag="sr")
            nc.vector.tensor_mul(sr[:], nsq[:], y1[:])
            f = itp.tile([BL, OA], F32, tag="f")
            nc.vector.tensor_mul(f[:], sr[:], rn1[:])
            vt = itp.tile([BL, N], F32 if t == NUM_ROUTING else BF16, tag="vt")
            nc.vector.tensor_mul(
                vt[:].rearrange("b (oa o) -> b oa o", o=O),
                s_t[:].rearrange("b (oa o) -> b oa o", o=O),
                f[:].unsqueeze(2).broadcast_to([BL, OA, O]),
            )
            return vt

        def delta(t, cb):
            """logits[cb] += sum_oa tmp via 16 accumulating id matmuls."""
            sl = slice(cb * CBS, (cb + 1) * CBS)
            dst = lg[cb][:, 0 : CBS * O]
            for oa in range(OA):
                nc.tensor.matmul(
                    dst,
                    lhsT=id128[:],
                    rhs=b4[:, sl, oa, :],
                    start=(t == 1 and oa == 0),
                    stop=(t == NUM_ROUTING - 1 and oa == OA - 1),
                    skip_group_check=True,
                )

        # ---- phase V ----
        with tc.tile_pool(name="psv", bufs=2, space="PSUM") as psv:
            # constants + act-table warm (Exp -> set 0, which also has Copy)
            nc.vector.memset(dz[:], 1.0)
            nc.scalar.activation(dl[:], dz[:], AF.Exp)
            nc.vector.memset(sh1[:], 1)
            nc.vector.memset(negm[:], -1)
            nc.vector.memset(magic[:], MAGIC)
            # DMA triggers cost ~630ns of queue time each, so spread them:
            # sync takes the w2c low half + smalls, scalar the high half,
            # vector the xfl waves.  GpSimd pre-zeroes w2c's off-diagonal
            # halves per wave (ranges disjoint from the DMAs, so parallel).
            for q in range(NDMA):
                sl = slice(q * DSTEP, (q + 1) * DSTEP)
                nc.gpsimd.memset(w2c[0:64, sl, N:N2], 0.0)
                nc.gpsimd.memset(w2c[64:P, sl, 0:N], 0.0)
            for q in range(NDMA):
                sl = slice(q * DSTEP, (q + 1) * DSTEP)
                nc.sync.dma_start(
                    out=w2c[0:64, sl, 0:N], in_=wc_d[0:64, sl, :]
                )
                nc.scalar.dma_start(
                    out=w2c[64:P, sl, N:N2], in_=wc_d[64:P, sl, :]
                )
                nc.sync.dma_start(out=xfl[:, sl, :], in_=xfl_d[:, sl, :])
                if q == 0:
                    nc.sync.dma_start(out=mask[:], in_=mask_d[:])
                elif q == 1:
                    nc.sync.dma_start(out=biasr[:], in_=bias_d[:])
                    nc.sync.dma_start(out=brep[:], in_=brep_d[:])
                elif q == 2:
                    nc.sync.dma_start(out=id128[:], in_=id_d[:])
                    nc.sync.dma_start(out=bsel[:], in_=bsel_d[:])
                # xbd wave: xfl broadcast over is' times the static mask
                nc.vector.tensor_mul(
                    xbd[:, sl, :].rearrange("p c (i b) -> p c i b", i=IS8),
                    xfl[:, sl, :].unsqueeze(2).broadcast_to(
                        [P, DSTEP, IS8, BL]
                    ),
                    mask[:].rearrange("p (i b) -> p i b", i=IS8)
                    .unsqueeze(1)
                    .broadcast_to([P, DSTEP, IS8, BL]),
                )

            def votes_pair(cp):
                # two votes matmuls per 2-bank psum tile, then one copy
                if cp % 2 == 0:
                    votes_pair.ps = psv.tile([P, 1024], F32, tag="pv")
                ps = votes_pair.ps
                j = cp % 2
                nc.tensor.matmul(
                    ps[:, j * 512 : j * 512 + N2],
                    lhsT=xbd[:, cp, :],
                    rhs=w2c[:, cp, :],
                    start=True,
                    stop=True,
                )
                if cp % 2 == 1:
                    g = cp // 2
                    src = ps[:].rearrange("p (j s) -> p j s", j=2)[:, :, 0:N2]
                    dst = votes[:, (cp - 1) * 2 : (cp + 1) * 2, :].rearrange(
                        "p (j c2) n -> p j (c2 n)", j=2
                    )
                    if g % 3 == 2:
                        nc.vector.tensor_copy(dst, src)
                    else:
                        nc.scalar.copy(dst, src)

            for cp in range(CP):
                nc.tensor.matmul(
                    s_ps[:, 0:N2],
                    lhsT=xfl[:, cp, :],
                    rhs=w2c[:, cp, :],
                    start=(cp == 0),
                    stop=(cp == CP - 1),
                )
                if cp >= LAG:
                    votes_pair(cp - LAG)
            v1 = squash(1)
            # half the flush, then the vrep matmul (so it runs as soon as
            # squash1 lands instead of behind the whole flush), then the rest
            for cp in range(CP - LAG, CP - LAG // 2):
                votes_pair(cp)
            vrep = itp.tile([P, N], BF16, tag="vrep")
            vr_ps1 = psv.tile([P, 1024], F32, tag="pv")
            nc.tensor.matmul(
                vr_ps1[:, 0:N], lhsT=brep[:], rhs=v1[:], start=True, stop=True
            )
            nc.scalar.copy(vrep[:], vr_ps1[:, 0:N])
            for cp in range(CP - LAG // 2, CP):
                votes_pair(cp)

        piv = ctx.enter_context(tc.tile_pool(name="piv", bufs=1, space="PSUM"))

        # ---- t=1: tmp = votes*vrep + delta into logits PSUM ----
        vr_b = vrep[:].unsqueeze(1).broadcast_to([P, C, N])
        for cb in range(CBN):
            for h in range(2):
                lo = cb * CBS + h * HW2
                nc.vector.tensor_mul(
                    big[:, lo : lo + HW2, :],
                    votes[:, lo : lo + HW2, :],
                    vr_b[:, lo : lo + HW2, :],
                )
            delta(1, cb)

        # ---- routing iterations t=2..3 ----
        for t in range(2, NUM_ROUTING + 1):
            r4 = route[:].unsqueeze(2).broadcast_to([P, C, OA, O])
            for cb in range(CBN):
                sl = slice(cb * CBS, (cb + 1) * CBS)
                src = lg[cb][:, 0 : CBS * O].rearrange("p (c o) -> p c o", o=O)
                nc.scalar.activation(expb[:, sl], src, AF.Exp)
                nc.vector.reduce_sum(z[:, sl], expb[:, sl], axis=AX.X)
                nc.vector.reciprocal_approx_fast(rz[:, sl], z[:, sl])
                nc.vector.tensor_mul(
                    route[:, sl],
                    expb[:, sl],
                    rz[:, sl].unsqueeze(2).broadcast_to([P, CBS, O]),
                )
                for h in range(2):
                    lo = cb * CBS + h * HW2
                    nc.vector.tensor_mul(
                        b4[:, lo : lo + HW2], v4[:, lo : lo + HW2],
                        r4[:, lo : lo + HW2],
                    )
                    for j in range(lo // SW, (lo + HW2) // SW):
                        rhs = big[:, j * SW : (j + 1) * SW, :].rearrange(
                            "p c n -> p (c n)"
                        )
                        nc.tensor.matmul(
                            s_ps[:, 0 : SW * N],
                            lhsT=bsel[:],
                            rhs=rhs,
                            start=(j == 0),
                            stop=(j == C // SW - 1),
                        )

            vt = squash(t)
            if t == NUM_ROUTING:
                nc.sync.dma_start(out=vout_d[:], in_=vt[:])
                break

            vrep2 = itp.tile([P, N], BF16, tag="vrep2")
            vr_ps = piv.tile([P, 512], F32, tag="vrps")
            nc.tensor.matmul(
                vr_ps[:, 0:N], lhsT=brep[:], rhs=vt[:], start=True, stop=True
            )
            nc.scalar.copy(vrep2[:], vr_ps[:, 0:N])

            vr_b2 = vrep2[:].unsqueeze(1).broadcast_to([P, C, N])
            for cb in range(CBN):
                for h in range(2):
                    lo = cb * CBS + h * HW2
                    nc.vector.tensor_mul(
                        big[:, lo : lo + HW2, :],
                        votes[:, lo : lo + HW2, :],
                        vr_b2[:, lo : lo + HW2, :],
                    )
                delta(t, cb)

    nc.compile()
    return nc


def get_nc():
    if "nc" not in _NC_CACHE:
        _NC_CACHE["nc"] = _build_nc()
    return _NC_CACHE["nc"]


def make_in_maps(x, weights, biases):
    bf = ml_dtypes.bfloat16
    x = np.asarray(x, np.float32)
    weights = np.asarray(weights, np.float32)
    biases = np.asarray(biases, np.float32)

    # wc[(h, is, a), cp, n=oa*O+o] = weights[(2cp+h)*8+is, a, o*OA+oa]
    w5 = (
        weights.reshape(CP, 2, IS8, A, O, OA)
        .transpose(0, 1, 2, 3, 5, 4)
        .reshape(CP, 2, IS8, A, N)
    )
    wc = np.ascontiguousarray(
        w5.transpose(1, 2, 3, 0, 4).reshape(P, CP, N)
    ).astype(bf)

    pidx = np.arange(P)
    is_p = (pidx // A) % IS8          # is of partition (h, is, a)
    is_c = np.arange(P) // BL         # is' of column (is', b)
    mask = (is_p[:, None] == is_c[None, :]).astype(np.float32).astype(bf)

    eye = np.eye(BL, dtype=np.float32)
    bsel = np.tile(eye, (IS8, 1)).astype(bf)  # bsel[p, b'] = delta(p % BL == b')
    brep = np.tile(eye, (1, IS8)).astype(bf)  # brep[b, p] = delta(b == p % BL)
    biasr = np.broadcast_to(biases.T.reshape(1, N), (BL, N)).astype(np.float32).copy()
    id128 = np.eye(P, dtype=np.float32).astype(bf)

    in_maps = []
    for k in range(NCORES):
        xc = x[k * BL : (k + 1) * BL]  # [BL, I, A]
        # xfl[(h, is, a), cp, b] = x[b, (2cp+h)*8+is, a]
        xfl = (
            xc.reshape(BL, CP, 2, IS8, A)
            .transpose(2, 3, 4, 1, 0)
            .reshape(P, CP, BL)
            .astype(bf)
        )
        in_maps.append(
            {
                "wc": wc,
                "xfl": np.ascontiguousarray(xfl),
                "mask": mask,
                "bsel": bsel,
                "brep": brep,
                "biasr": biasr,
                "id128": id128,
            }
        )
    return in_maps


def assemble_out(results):
    out = np.zeros((B, 1, O, OA), np.float32)
    for k in range(NCORES):
        v = np.asarray(results[k]["vout"], np.float32)  # [BL, N], n = oa*O + o
        out[k * BL : (k + 1) * BL, 0] = v.reshape(BL, OA, O).transpose(0, 2, 1)
    return out


def kernel(x, weights, biases):
    from concourse.bass_utils import run_bass_kernel_spmd

    nc = get_nc()
    in_maps = make_in_maps(x, weights, biases)
    res = run_bass_kernel_spmd(nc, in_maps, list(range(NCORES)))
    return assemble_out(res.results)


# revision 19
# speedup vs baseline: 1.2926x; 1.0458x over previous
"""CapsuleLayer (dynamic routing) Trainium2 kernel, v4.

Problem: B=128, I=1152 input capsules (A=8), O=10 output capsules (OA=16),
3 routing iterations.  Data-parallel over batch: 8 cores x 16 examples.

Per-core layout: SBUF partition p = is*16 + b (is = i mod 8, b = local batch),
half-chunk c = i // 8 in the free dim, vote coordinate n = oa*10 + o.
Phase V contracts over q = (h, is, a) (128 rows) per PAIR of half-chunks
(f=320 per matmul so the ~120ns LDWEIGHTS hides under the stream).

v4 (vs v2 baseline):
  - Compact weight DMA: w2c's block-diagonal zero halves are memset by the
    idle GpSimd engine; DMA ships only the 2.95MB compact weights (two
    strided halves) instead of the 5.9MB doubled tensor.  xbd is built on
    DVE as xfl*mask instead of DMAing the 16x-inflated 2.36MB tensor.
    Total input DMA: 8.4MB -> 3.2MB.
  - s1 matmuls run LAG pair-chunks ahead of the votes matmuls so squash1 /
    vrep / t=1 tmp start as early as possible.
  - PSUM->SBUF votes copies on Scalar(2/3) + Vector(1/3) with 2x2-bank
    PSUM buffers; no PE stalls.
  - Squash needs no Scalar tables: sqrt via DVE fast-inverse-sqrt
    (bit-trick seed + 1 Newton step).  Only Exp (softmax) and Copy (vrep)
    remain on Scalar -> a single ACT_TABLE_LOAD at t=0, warmed by a dummy.
  - No GpSimd elementwise offload: Pool TENSOR_TENSOR is ~3.9ns/el AND
    SBUF port contention halves DVE to 1x when they overlap (measured).
"""

import numpy as np
import ml_dtypes

B, I, A, O, OA = 128, 1152, 8, 10, 16
NCORES = 8
BL = B // NCORES        # 16 examples per core
IS8 = 8                 # i-positions per half-chunk
C = I // IS8            # 144 half-chunks
CP = C // 2             # 72 paired chunks
N = O * OA              # 160, n = oa*O + o
N2 = 2 * N              # 320 per paired chunk
P = 128                 # p = is*BL + b
NUM_ROUTING = 3
CBN = 3                 # logits c-blocks (PSUM banks)
CBS = C // CBN          # 48 half-chunks per block
HW2 = CBS // 2          # 24 half-chunks per wave
SW = 3                  # half-chunks per s-matmul
LAG = 12                # votes-mm lag behind s1-mm, in paired chunks
NDMA = 6                # input DMA waves
DSTEP = CP // NDMA      # 12 paired chunks per wave

MAGIC = 0x5F3759DF      # fast inverse sqrt seed

_NC_CACHE = {}


def _build_nc():
    from contextlib import ExitStack

    import concourse.tile as tile
    import concourse.mybir as mybir
    from concourse import bacc

    F32 = mybir.dt.float32
    I32 = mybir.dt.int32
    BF16 = mybir.dt.bfloat16
    AF = mybir.ActivationFunctionType
    ALU = mybir.AluOpType
    AX = mybir.AxisListType

    nc = bacc.Bacc()
    w2c_d = nc.dram_tensor("w2c", [P, CP, N2], BF16, kind="ExternalInput")
    xfl_d = nc.dram_tensor("xfl", [P, CP, BL], BF16, kind="ExternalInput")
    mask_d = nc.dram_tensor("mask", [P, P], BF16, kind="ExternalInput")
    bsel_d = nc.dram_tensor("bsel", [P, BL], BF16, kind="ExternalInput")
    brep_d = nc.dram_tensor("brep", [BL, P], BF16, kind="ExternalInput")
    bias_d = nc.dram_tensor("biasr", [BL, N], F32, kind="ExternalInput")
    id_d = nc.dram_tensor("id128", [P, P], BF16, kind="ExternalInput")
    vout_d = nc.dram_tensor("vout", [BL, N], F32, kind="ExternalOutput")

    with ExitStack() as ctx:
        tc = ctx.enter_context(tile.TileContext(nc))
        st = ctx.enter_context(tc.tile_pool(name="static", bufs=1))
        itp = ctx.enter_context(tc.tile_pool(name="itp", bufs=1))
        pps = ctx.enter_context(tc.tile_pool(name="pps", bufs=1, space="PSUM"))

        w2c = st.tile([P, CP, N2], BF16)
        xbd = st.tile([P, CP, P], BF16)
        xfl = st.tile([P, CP, BL], BF16)
        mask = st.tile([P, P], BF16)
        votes = st.tile([P, C, N], BF16)
        big = st.tile([P, C, N], BF16)      # shared wv/tmp buffer
        expb = st.tile([P, C, O], BF16)
        route = st.tile([P, C, O], BF16)
        z = st.tile([P, C], F32)
        rz = st.tile([P, C], F32)
        bsel = st.tile([P, BL], BF16)
        brep = st.tile([BL, P], BF16)
        biasr = st.tile([BL, N], F32)
        id128 = st.tile([P, P], BF16)
        sh1 = st.tile([BL, 1], I32)
        negm = st.tile([BL, 1], I32)
        magic = st.tile([BL, 1], I32)
        dz = st.tile([1, 1], F32)
        dl = st.tile([1, 1], F32)

        # PSUM (bank = 512 f32): 3 logits banks + 1 s bank (persistent)
        lg0 = pps.tile([P, 512], F32, tag="lg0")
        lg1 = pps.tile([P, 512], F32, tag="lg1")
        lg2 = pps.tile([P, 512], F32, tag="lg2")
        lg = [lg0, lg1, lg2]
        s_ps = pps.tile([BL, 512], F32, tag="sps")

        v4 = votes[:].rearrange("p c (oa o) -> p c oa o", o=O)
        b4 = big[:].rearrange("p c (oa o) -> p c oa o", o=O)

        def squash(t):
            """squash s -> v tile (DVE only: fast-inverse-sqrt, no tables)."""
            s_t = itp.tile([BL, N], F32, tag="stile")
            if t == 1:
                # s1 psum holds the two i-parity halves side by side
                # (one PSUM operand max per DVE op: copy one half out first)
                sa = itp.tile([BL, N], F32, tag="sa")
                nc.vector.tensor_copy(sa[:], s_ps[:, 0:N])
                nc.vector.tensor_add(sa[:], sa[:], s_ps[:, N:N2])
                nc.vector.scalar_tensor_tensor(
                    s_t[:], sa[:], 1.0 / O, biasr[:], op0=ALU.mult, op1=ALU.add
                )
            else:
                sa = itp.tile([BL, N], F32, tag="sa")
                nc.vector.reduce_sum(
                    sa[:],
                    s_ps[:, 0 : SW * N].rearrange("b (c n) -> b n c", c=SW),
                    axis=AX.X,
                )
                nc.vector.tensor_add(s_t[:], sa[:], biasr[:])
            sq = itp.tile([BL, N], F32, tag="sq")
            nc.vector.tensor_mul(sq[:], s_t[:], s_t[:])
            nsq = itp.tile([BL, OA], F32, tag="nsq")
            nc.vector.reduce_sum(
                nsq[:], sq[:].rearrange("b (oa o) -> b oa o", o=O), axis=AX.X
            )
            nsq1 = itp.tile([BL, OA], F32, tag="nsq1")
            nc.vector.tensor_scalar_add(nsq1[:], nsq[:], 1.0)
            rn1 = itp.tile([BL, OA], F32, tag="rn1")
            nc.vector.reciprocal_approx_fast(rn1[:], nsq1[:])
            # sqrt(nsq) = nsq * rsqrt(nsq), rsqrt via bit-trick + 1 NR step
            # (int scalars aren't allowed on tensor_scalar, so the shift and
            # subtract use broadcast [BL,1] int tiles through tensor_tensor)
            hb = itp.tile([BL, OA], I32, tag="hb")
            nc.vector.tensor_tensor(
                hb[:], nsq[:].bitcast(I32),
                sh1[:].broadcast_to([BL, OA]),
                op=ALU.arith_shift_right,
            )
            y0 = itp.tile([BL, OA], I32, tag="y0")
            nc.vector.tensor_tensor(
                y0[:], magic[:].broadcast_to([BL, OA]), hb[:], op=ALU.subtract
            )
            y0f = y0[:].bitcast(F32)
            tt = itp.tile([BL, OA], F32, tag="tt")
            nc.vector.tensor_mul(tt[:], nsq[:], y0f)
            nc.vector.tensor_mul(tt[:], tt[:], y0f)
            uu = itp.tile([BL, OA], F32, tag="uu")
            nc.vector.tensor_scalar(
                uu[:], tt[:], -0.5, 1.5, op0=ALU.mult, op1=ALU.add
            )
            y1 = itp.tile([BL, OA], F32, tag="y1")
            nc.vector.tensor_mul(y1[:], y0f, uu[:])
            sr = itp.tile([BL, OA], F32, tag="sr")
            nc.vector.tensor_mul(sr[:], nsq[:], y1[:])
            f = itp.tile([BL, OA], F32, tag="f")
            nc.vector.tensor_mul(f[:], sr[:], rn1[:])
            vt = itp.tile([BL, N], F32 if t == NUM_ROUTING else BF16, tag="vt")
            nc.vector.tensor_mul(
                vt[:].rearrange("b (oa o) -> b oa o", o=O),
                s_t[:].rearrange("b (oa o) -> b oa o", o=O),
                f[:].unsqueeze(2).broadcast_to([BL, OA, O]),
            )
            return vt

        def delta(t, cb):
            """logits[cb] += sum_oa tmp via 16 accumulating id matmuls."""
            sl = slice(cb * CBS, (cb + 1) * CBS)
            dst = lg[cb][:, 0 : CBS * O]
            for oa in range(OA):
                nc.tensor.matmul(
                    dst,
                    lhsT=id128[:],
                    rhs=b4[:, sl, oa, :],
                    start=(t == 1 and oa == 0),
                    stop=(t == NUM_ROUTING - 1 and oa == OA - 1),
                    skip_group_check=True,
                )

        # ---- phase V ----
        with tc.tile_pool(name="psv", bufs=2, space="PSUM") as psv:
            # constants + act-table warm (Exp -> set 0, which also has Copy)
            nc.vector.memset(dz[:], 1.0)
            nc.scalar.activation(dl[:], dz[:], AF.Exp)
            nc.vector.memset(sh1[:], 1)
            nc.vector.memset(negm[:], -1)
            nc.vector.memset(magic[:], MAGIC)
            # DMA triggers cost ~630ns of queue time each: xfl/mask go first
            # (they gate the xbd builds which gate the votes matmuls), then
            # the w2c waves with the small tensors interleaved.
            nc.sync.dma_start(out=xfl[:], in_=xfl_d[:])
            nc.sync.dma_start(out=mask[:], in_=mask_d[:])
            for q in range(NDMA):
                sl = slice(q * DSTEP, (q + 1) * DSTEP)
                nc.sync.dma_start(out=w2c[:, sl, :], in_=w2c_d[:, sl, :])
                if q == 0:
                    nc.scalar.dma_start(out=biasr[:], in_=bias_d[:])
                    nc.scalar.dma_start(out=brep[:], in_=brep_d[:])
                elif q == 1:
                    nc.scalar.dma_start(out=id128[:], in_=id_d[:])
                    nc.scalar.dma_start(out=bsel[:], in_=bsel_d[:])
                # xbd wave: xfl broadcast over is' times the static mask
                nc.vector.tensor_mul(
                    xbd[:, sl, :].rearrange("p c (i b) -> p c i b", i=IS8),
                    xfl[:, sl, :].unsqueeze(2).broadcast_to(
                        [P, DSTEP, IS8, BL]
                    ),
                    mask[:].rearrange("p (i b) -> p i b", i=IS8)
                    .unsqueeze(1)
                    .broadcast_to([P, DSTEP, IS8, BL]),
                )

            def votes_pair(cp):
                # two votes matmuls per 2-bank psum tile, then one copy
                if cp % 2 == 0:
                    votes_pair.ps = psv.tile([P, 1024], F32, tag="pv")
                ps = votes_pair.ps
                j = cp % 2
                nc.tensor.matmul(
                    ps[:, j * 512 : j * 512 + N2],
                    lhsT=xbd[:, cp, :],
                    rhs=w2c[:, cp, :],
                    start=True,
                    stop=True,
                )
                if cp % 2 == 1:
                    g = cp // 2
                    src = ps[:].rearrange("p (j s) -> p j s", j=2)[:, :, 0:N2]
                    dst = votes[:, (cp - 1) * 2 : (cp + 1) * 2, :].rearrange(
                        "p (j c2) n -> p j (c2 n)", j=2
                    )
                    if g % 3 == 2 and g < 24:
                        nc.vector.tensor_copy(dst, src)
                    else:
                        nc.scalar.copy(dst, src)

            for cp in range(CP):
                nc.tensor.matmul(
                    s_ps[:, 0:N2],
                    lhsT=xfl[:, cp, :],
                    rhs=w2c[:, cp, :],
                    start=(cp == 0),
                    stop=(cp == CP - 1),
                )
                if cp >= LAG:
                    votes_pair(cp - LAG)
            v1 = squash(1)
            # half the flush, then the vrep matmul (so it runs as soon as
            # squash1 lands instead of behind the whole flush), then the rest
            for cp in range(CP - LAG, CP - LAG // 2):
                votes_pair(cp)
            vrep = itp.tile([P, N], BF16, tag="vrep")
            vr_ps1 = psv.tile([P, 1024], F32, tag="pv")
            nc.tensor.matmul(
                vr_ps1[:, 0:N], lhsT=brep[:], rhs=v1[:], start=True, stop=True
            )
            nc.scalar.copy(vrep[:], vr_ps1[:, 0:N])
            for cp in range(CP - LAG // 2, CP):
                votes_pair(cp)

        piv = ctx.enter_context(tc.tile_pool(name="piv", bufs=1, space="PSUM"))

        # ---- t=1: tmp = votes*vrep + delta into logits PSUM ----
        vr_b = vrep[:].unsqueeze(1).broadcast_to([P, C, N])
        for cb in range(CBN):
            for h in range(2):
                lo = cb * CBS + h * HW2
                nc.vector.tensor_mul(
                    big[:, lo : lo + HW2, :],
                    votes[:, lo : lo + HW2, :],
                    vr_b[:, lo : lo + HW2, :],
                )
            delta(1, cb)

        # ---- routing iterations t=2..3 ----
        for t in range(2, NUM_ROUTING + 1):
            r4 = route[:].unsqueeze(2).broadcast_to([P, C, OA, O])
            for cb in range(CBN):
                sl = slice(cb * CBS, (cb + 1) * CBS)
                src = lg[cb][:, 0 : CBS * O].rearrange("p (c o) -> p c o", o=O)
                nc.scalar.activation(expb[:, sl], src, AF.Exp)
                nc.vector.reduce_sum(z[:, sl], expb[:, sl], axis=AX.X)
                nc.vector.reciprocal_approx_fast(rz[:, sl], z[:, sl])
                nc.vector.tensor_mul(
                    route[:, sl],
                    expb[:, sl],
                    rz[:, sl].unsqueeze(2).broadcast_to([P, CBS, O]),
                )
                for h in range(2):
                    lo = cb * CBS + h * HW2
                    nc.vector.tensor_mul(
                        b4[:, lo : lo + HW2], v4[:, lo : lo + HW2],
                        r4[:, lo : lo + HW2],
                    )
                    for j in range(lo // SW, (lo + HW2) // SW):
                        rhs = big[:, j * SW : (j + 1) * SW, :].rearrange(
                            "p c n -> p (c n)"
                        )
                        nc.tensor.matmul(
                            s_ps[:, 0 : SW * N],
                            lhsT=bsel[:],
                            rhs=rhs,
                            start=(j == 0),
                            stop=(j == C // SW - 1),
                        )

            vt = squash(t)
            if t == NUM_ROUTING:
                nc.sync.dma_start(out=vout_d[:], in_=vt[:])
                break

            vrep2 = itp.tile([P, N], BF16, tag="vrep2")
            vr_ps = piv.tile([P, 512], F32, tag="vrps")
            nc.tensor.matmul(
                vr_ps[:, 0:N], lhsT=brep[:], rhs=vt[:], start=True, stop=True
            )
            nc.scalar.copy(vrep2[:], vr_ps[:, 0:N])

            vr_b2 = vrep2[:].unsqueeze(1).broadcast_to([P, C, N])
            for cb in range(CBN):
                for h in range(2):
                    lo = cb * CBS + h * HW2
                    nc.vector.tensor_mul(
                        big[:, lo : lo + HW2, :],
                        votes[:, lo : lo + HW2, :],
                        vr_b2[:, lo : lo + HW2, :],
                    )
                delta(t, cb)

    nc.compile()
    return nc


def get_nc():
    if "nc" not in _NC_CACHE:
        _NC_CACHE["nc"] = _build_nc()
    return _NC_CACHE["nc"]


def make_in_maps(x, weights, biases):
    bf = ml_dtypes.bfloat16
    x = np.asarray(x, np.float32)
    weights = np.asarray(weights, np.float32)
    biases = np.asarray(biases, np.float32)

    # w2c[(h,is,a), cp, h2*N+n] = w[(2cp+h)*8+is, a, n] * (h==h2)
    w5 = (
        weights.reshape(CP, 2, IS8, A, O, OA)
        .transpose(0, 1, 2, 3, 5, 4)
        .reshape(CP, 2, IS8, A, N)
    )
    w2cf = np.zeros((CP, 2, IS8, A, 2, N), np.float32)
    for hh in range(2):
        w2cf[:, hh, :, :, hh, :] = w5[:, hh]
    w2c = w2cf.reshape(CP, P, N2).transpose(1, 0, 2).astype(bf)

    pidx = np.arange(P)
    is_p = (pidx // A) % IS8          # is of partition (h, is, a)
    is_c = np.arange(P) // BL         # is' of column (is', b)
    mask = (is_p[:, None] == is_c[None, :]).astype(np.float32).astype(bf)

    eye = np.eye(BL, dtype=np.float32)
    bsel = np.tile(eye, (IS8, 1)).astype(bf)  # bsel[p, b'] = delta(p % BL == b')
    brep = np.tile(eye, (1, IS8)).astype(bf)  # brep[b, p] = delta(b == p % BL)
    biasr = np.broadcast_to(biases.T.reshape(1, N), (BL, N)).astype(np.float32).copy()
    id128 = np.eye(P, dtype=np.float32).astype(bf)

    in_maps = []
    for k in range(NCORES):
        xc = x[k * BL : (k + 1) * BL]  # [BL, I, A]
        # xfl[(h, is, a), cp, b] = x[b, (2cp+h)*8+is, a]
        xfl = (
            xc.reshape(BL, CP, 2, IS8, A)
            .transpose(2, 3, 4, 1, 0)
            .reshape(P, CP, BL)
            .astype(bf)
        )
        in_maps.append(
            {
                "w2c": np.ascontiguousarray(w2c),
                "xfl": np.ascontiguousarray(xfl),
                "mask": mask,
                "bsel": bsel,
                "brep": brep,
                "biasr": biasr,
                "id128": id128,
            }
        )
    return in_maps


def assemble_out(results):
    out = np.zeros((B, 1, O, OA), np.float32)
    for k in range(NCORES):
        v = np.asarray(results[k]["vout"], np.float32)  # [BL, N], n = oa*O + o
        out[k * BL : (k + 1) * BL, 0] = v.reshape(BL, OA, O).transpose(0, 2, 1)
    return out


def kernel(x, weights, biases):
    from concourse.bass_utils import run_bass_kernel_spmd

    nc = get_nc()
    in_maps = make_in_maps(x, weights, biases)
    res = run_bass_kernel_spmd(nc, in_maps, list(range(NCORES)))
    return assemble_out(res.results)
